# revision 11
# baseline (speedup 1.0000x reference)
"""Trainium2 Bass kernel for nn_DecoderLayer (B=16,S=512,D=512,H=8).

Sharding: pure data-parallel over batch. 16 batches / 8 cores = 2 per core.
Each core runs both attention blocks + output projection for its 2 batches.

Layout strategy (all matmuls keep the contraction dim on SBUF partitions):
  - inputs de_x/en_x are loaded natively then PE-transposed to x^T [d, s]
  - q^T/k^T computed per head-pair [128, 512] (two heads stacked on partitions)
  - v computed natively [t, e] for all heads at once (N=512 moving dim)
  - scores are computed transposed (p^T[t, s]) so softmax'd weights are
    directly usable as the moving operand of the PV matmul
  - softmax has no max-subtraction (scores ~ N(0,1), exp is safe in fp32);
    causal mask applied with affine_select (fill=0 after exp)
  - PV uses lhsT=[v | ones] so the softmax denominator Z lands in psum row 64
  - normalization (1/Z) is broadcast across partitions with a K=1 matmul
  - h1^T / h2^T are assembled per d-tile so the next projection can consume
    them directly as stationary operands (no extra transposes)
All matmul-feeding tensors are fp32r end-to-end (full PE rate at N>=256);
PSUM accumulation stays fp32.
"""

import numpy as np
from contextlib import ExitStack

import concourse.bacc as bacc
import concourse.bass as bass
import concourse.mybir as mybir
import concourse.tile as tile
from concourse.bass_utils import run_bass_kernel_spmd
from concourse.masks import make_identity

B, S, D, H = 16, 512, 512, 8
DH = D // H              # 64
NCORES = 8
BPC = B // NCORES        # 2 batches per core
P = 128
NT = S // P              # 4 tiles along s/t/d
F32 = mybir.dt.float32
F32R = mybir.dt.float32r
EXP = mybir.ActivationFunctionType.Exp
MULT = mybir.AluOpType.mult
ADD = mybir.AluOpType.add
GE = mybir.AluOpType.is_ge


def _build():
    nc = bacc.Bacc("TRN2", target_bir_lowering=False)
    de = nc.dram_tensor("de_x", [BPC, S, D], F32R, kind="ExternalInput")
    en = nc.dram_tensor("en_x", [BPC, S, D], F32R, kind="ExternalInput")
    wq = nc.dram_tensor("wq", [D, D], F32R, kind="ExternalInput")
    wk = nc.dram_tensor("wk", [D, D], F32R, kind="ExternalInput")
    wv = nc.dram_tensor("wv", [D, D], F32R, kind="ExternalInput")
    w2 = nc.dram_tensor("w2", [D, D], F32R, kind="ExternalInput")
    b2 = nc.dram_tensor("b2", [1, D], F32R, kind="ExternalInput")
    out = nc.dram_tensor("out", [BPC, S, D], F32, kind="ExternalOutput")

    with tile.TileContext(nc) as tc:
        with ExitStack() as ctx:
            _emit(ctx, tc, nc, de, en, wq, wk, wv, w2, b2, out)
    nc.finalize()
    return nc


def _emit(ctx, tc, nc, de, en, wq, wk, wv, w2, b2, out):
    const = ctx.enter_context(tc.tile_pool(name="const", bufs=1))
    xtp = ctx.enter_context(tc.tile_pool(name="xtp", bufs=1))
    qkp = ctx.enter_context(tc.tile_pool(name="qkp", bufs=1))
    vsp = ctx.enter_context(tc.tile_pool(name="vsp", bufs=1))
    htp = ctx.enter_context(tc.tile_pool(name="htp", bufs=1))
    natp = ctx.enter_context(tc.tile_pool(name="natp", bufs=3))
    etp = ctx.enter_context(tc.tile_pool(name="etp", bufs=8))
    rowp = ctx.enter_context(tc.tile_pool(name="rowp", bufs=4))
    stgp = ctx.enter_context(tc.tile_pool(name="stgp", bufs=2))
    outp = ctx.enter_context(tc.tile_pool(name="outp", bufs=2))
    ps = ctx.enter_context(tc.tile_pool(name="ps", bufs=8, space="PSUM"))

    # --- one-time constants ---
    # Memset can't write fp32r directly (invalid ISA value type), so consts
    # are built in an fp32 scratch and rounded into fp32r via DVE copies.
    scr = const.tile([P, 3 * P], F32, tag="scr", name="scr")
    zeros_r = const.tile([P, 3 * P], F32R, tag="zeros", name="zeros_r")
    nc.gpsimd.memset(scr, 0.0)
    nc.vector.tensor_copy(zeros_r, scr)
    ident = const.tile([P, P], F32R, tag="ident", name="ident")
    make_identity(nc, scr[:, 0:P], nomemset=True)
    nc.vector.tensor_copy(ident, scr[:, 0:P])
    ones_t = const.tile([P, P], F32R, tag="ones", name="ones")
    nc.gpsimd.memset(scr[:, 0:P], 1.0)
    nc.vector.tensor_copy(ones_t, scr[:, 0:P])

    b2row = const.tile([1, D], F32R, tag="b2row", name="b2row")
    nc.sync.dma_start(b2row, b2[0:1, :])
    pb2 = ps.tile([P, D], F32, tag="ps", name="psmm")
    nc.tensor.matmul(pb2, (ones_t[0:1, 0:P]), (b2row), start=True, stop=True)
    b2t = const.tile([P, D], F32, tag="b2t", name="b2t")
    nc.scalar.copy(b2t, pb2)

    w_sb = {}
    for name, dram in (("wq", wq), ("wk", wk), ("wv", wv), ("w2", w2)):
        tiles = []
        for dt in range(NT):
            t = const.tile([P, D], F32R, tag=f"{name}{dt}", name=f"w_{name}{dt}")
            nc.sync.dma_start(t, dram[dt * P:(dt + 1) * P, :])
            tiles.append(t)
        w_sb[name] = tiles

    for b in range(BPC):
        # --- phase A: load inputs and build x^T [d, s] ---
        xts = {}
        for name, dram in (("de", de), ("en", en)):
            xt = [xtp.tile([P, S], F32R, tag=f"{name}T{dt}", name=f"{name}T{dt}") for dt in range(NT)]
            for st in range(NT):
                natt = natp.tile([P, D], F32R, tag="nat", name="nat")
                nc.sync.dma_start(natt, dram[b, st * P:(st + 1) * P, :])
                for dt in range(NT):
                    pt = ps.tile([P, P], F32R, tag="ps", name="pst")
                    nc.tensor.transpose(pt, natt[:, dt * P:(dt + 1) * P], ident)
                    nc.vector.tensor_copy(xt[dt][:, st * P:(st + 1) * P], pt)
            xts[name] = xt

        def qk_proj(xt, wname, tagpfx):
            dst = []
            for hp in range(4):
                pq = ps.tile([P, D], F32, tag="ps", name="psmm")
                for dt in range(NT):
                    nc.tensor.matmul(
                        pq,
                        (w_sb[wname][dt][:, hp * P:(hp + 1) * P]),
                        (xt[dt]),
                        start=dt == 0,
                        stop=dt == NT - 1,
                    )
                t = qkp.tile([P, D], F32R, tag=f"{tagpfx}{hp}", name=f"{tagpfx}{hp}")
                nc.scalar.copy(t, pq)
                dst.append(t)
            return dst

        def v_proj(lhsT_tiles, tagpfx):
            # native [t, e] values for all heads; layout [128, 8*65] with a
            # ones column per head (for the softmax denominator)
            dst = []
            for tt in range(NT):
                pv = ps.tile([P, D], F32, tag="ps", name="psmm")
                for dt in range(NT):
                    nc.tensor.matmul(
                        pv,
                        (lhsT_tiles[dt][:, tt * P:(tt + 1) * P]),
                        (w_sb["wv"][dt]),
                        start=dt == 0,
                        stop=dt == NT - 1,
                    )
                t = vsp.tile([P, H * (DH + 1)], F32R, tag=f"{tagpfx}{tt}", name=f"{tagpfx}{tt}")
                dv = t.rearrange("p (h x) -> p h x", x=DH + 1)
                nc.vector.tensor_copy(
                    dv[:, :, 0:DH], pv.rearrange("p (h e) -> p h e", e=DH)
                )
                nc.vector.tensor_copy(
                    dv[:, :, DH:DH + 1],
                    ones_t[:, 0:H].rearrange("p (h o) -> p h o", o=1),
                )
                dst.append(t)
            return dst

        def attn_head(h, qT, kT, v_s, hT, causal):
            hp, odd = divmod(h, 2)
            off = DH * odd
            e_tiles = []
            for ti in range(NT):
                s0 = ti * P if causal else 0
                pp = ps.tile([P, D], F32, tag="ps", name="psmm")
                nc.tensor.matmul(
                    pp[:, s0:D],
                    (kT[hp][off:off + DH, ti * P:(ti + 1) * P]),
                    (qT[hp][off:off + DH, s0:D]),
                    start=True,
                    stop=True,
                )
                et = etp.tile([P, D], F32R, tag="et", name="et")
                if s0 > 0:
                    nc.vector.tensor_copy(et[:, 0:s0], zeros_r[:, 0:s0])
                nc.scalar.activation(et[:, s0:D], pp[:, s0:D], EXP, scale=0.125)
                if causal:
                    nc.gpsimd.affine_select(
                        out=et[:, s0:s0 + P],
                        in_=et[:, s0:s0 + P],
                        compare_op=GE,
                        fill=0.0,
                        base=0,
                        pattern=[[1, P]],
                        channel_multiplier=-1,
                    )
                e_tiles.append(et)

            pa = ps.tile([DH + 1, D], F32, tag="ps", name="pa")
            for ti in range(NT):
                nc.tensor.matmul(
                    pa,
                    (v_s[ti][:, h * (DH + 1):(h + 1) * (DH + 1)]),
                    (e_tiles[ti]),
                    start=ti == 0,
                    stop=ti == NT - 1,
                )
            # 1/Z broadcast across partitions via K=1 matmul
            rq = rowp.tile([DH + 1, D], F32R, tag="rq", name="rq")
            with nc.allow_low_precision("fp32r feeds the PE, full fp32 kept in PSUM"):
                nc.vector.reciprocal(rq[DH:DH + 1, :], pa[DH:DH + 1, :])
            prb = ps.tile([DH, D], F32, tag="ps", name="prb")
            nc.tensor.matmul(
                prb,
                (ones_t[DH:DH + 1, 0:DH]),
                (rq[DH:DH + 1, :]),
                start=True,
                stop=True,
            )
            # DVE can't read two PSUM operands (one PSUM read port) — evict
            # the broadcast to SBUF first
            prb_sb = stgp.tile([DH, D], F32, tag="prb", name="prb_sb")
            nc.scalar.copy(prb_sb, prb)
            if not odd:
                nc.vector.tensor_tensor(hT[hp][0:DH, :], pa[0:DH, :], prb_sb, MULT)
            else:
                stg = stgp.tile([DH, D], F32R, tag="stg", name="stg")
                nc.vector.tensor_tensor(stg, pa[0:DH, :], prb_sb, MULT)
                # partition shift (rows 0-63 -> 64-127) via SBUF->SBUF DMA
                nc.sync.dma_start(hT[hp][DH:P, :], stg)

        # --- block 1: masked self-attention on de_x ---
        q1T = qk_proj(xts["de"], "wq", "q1T")
        k1T = qk_proj(xts["de"], "wk", "k1T")
        v1s = v_proj(xts["de"], "v1s")
        h1T = [htp.tile([P, S], F32R, tag=f"h1T{dt}", name=f"h1T{dt}") for dt in range(NT)]
        for h in range(H):
            attn_head(h, q1T, k1T, v1s, h1T, causal=True)

        # --- block 2: same weights, q=k=en_x, v=h1 ---
        q2T = qk_proj(xts["en"], "wq", "q2T")
        k2T = qk_proj(xts["en"], "wk", "k2T")
        v2s = v_proj(h1T, "v2s")
        h2T = [htp.tile([P, S], F32R, tag=f"h2T{dt}", name=f"h2T{dt}") for dt in range(NT)]
        for h in range(H):
            attn_head(h, q2T, k2T, v2s, h2T, causal=False)

        # --- output projection + bias ---
        for st in range(NT):
            po = ps.tile([P, D], F32, tag="ps", name="psmm")
            for dt in range(NT):
                nc.tensor.matmul(
                    po,
                    (h2T[dt][:, st * P:(st + 1) * P]),
                    (w_sb["w2"][dt]),
                    start=dt == 0,
                    stop=dt == NT - 1,
                )
            ot = outp.tile([P, D], F32, tag="ot", name="ot")
            nc.vector.tensor_tensor(ot, po, b2t, ADD)
            nc.sync.dma_start(out[b, st * P:(st + 1) * P, :], ot)


def kernel(de_x, en_x, mask, Wq, Wk, Wv, W2, b2, _trace=False):
    de_x = np.asarray(de_x, dtype=np.float32)
    en_x = np.asarray(en_x, dtype=np.float32)
    # weights [H, D, DH] -> flat [D, H*DH]
    wqf = np.ascontiguousarray(np.transpose(np.asarray(Wq, np.float32), (1, 0, 2)).reshape(D, D))
    wkf = np.ascontiguousarray(np.transpose(np.asarray(Wk, np.float32), (1, 0, 2)).reshape(D, D))
    wvf = np.ascontiguousarray(np.transpose(np.asarray(Wv, np.float32), (1, 0, 2)).reshape(D, D))
    w2f = np.ascontiguousarray(np.asarray(W2, np.float32))
    b2f = np.ascontiguousarray(np.asarray(b2, np.float32).reshape(1, D))

    nc = _build()
    in_maps = []
    for c in range(NCORES):
        in_maps.append({
            "de_x": np.ascontiguousarray(de_x[c * BPC:(c + 1) * BPC]),
            "en_x": np.ascontiguousarray(en_x[c * BPC:(c + 1) * BPC]),
            "wq": wqf, "wk": wkf, "wv": wvf, "w2": w2f, "b2": b2f,
        })
    res = run_bass_kernel_spmd(nc, in_maps, list(range(NCORES)), trace=_trace)
    outs = np.concatenate([res.results[c]["out"] for c in range(NCORES)], axis=0)
    if _trace:
        return outs, res
    return outs


# revision 13
# speedup vs baseline: 7.2407x; 7.2407x over previous
"""Trainium2 Bass kernel for nn_DecoderLayer (B=16,S=512,D=512,H=8).

Sharding: pure data-parallel over batch. 16 batches / 8 cores = 2 per core.
Each core runs both attention blocks + output projection for its 2 batches.

Layout strategy (all matmuls keep the contraction dim on SBUF partitions):
  - inputs de_x/en_x are loaded natively then PE-transposed to x^T [d, s]
  - q^T/k^T computed per head-pair [128, 512] (two heads stacked on partitions)
  - v computed natively [t, e] for all heads at once (N=512 moving dim)
  - scores are computed transposed (p^T[t, s]) so softmax'd weights are
    directly usable as the moving operand of the PV matmul
  - softmax has no max-subtraction (scores ~ N(0,1), exp is safe in fp32);
    causal mask applied with affine_select (fill=0 after exp)
  - PV uses lhsT=[v | ones] so the softmax denominator Z lands in psum row 64
  - normalization (1/Z) is broadcast across partitions with a K=1 matmul
  - h1^T / h2^T are assembled per d-tile so the next projection can consume
    them directly as stationary operands (no extra transposes)
All matmul-feeding tensors are fp32r end-to-end (full PE rate at N>=256);
PSUM accumulation stays fp32.
"""

import numpy as np
from contextlib import ExitStack

import concourse.bacc as bacc
import concourse.bass as bass
import concourse.mybir as mybir
import concourse.tile as tile
from concourse.bass_utils import run_bass_kernel_spmd
from concourse.masks import make_identity

B, S, D, H = 16, 512, 512, 8
DH = D // H              # 64
NCORES = 8
BPC = B // NCORES        # 2 batches per core
P = 128
NT = S // P              # 4 tiles along s/t/d
F32 = mybir.dt.float32
F32R = mybir.dt.float32r
EXP = mybir.ActivationFunctionType.Exp
MULT = mybir.AluOpType.mult
ADD = mybir.AluOpType.add
GE = mybir.AluOpType.is_ge


def _build(repeat=1):
    nc = bacc.Bacc("TRN2", target_bir_lowering=False)
    de = nc.dram_tensor("de_x", [BPC, S, D], F32R, kind="ExternalInput")
    en = nc.dram_tensor("en_x", [BPC, S, D], F32R, kind="ExternalInput")
    wq = nc.dram_tensor("wq", [D, D], F32R, kind="ExternalInput")
    wk = nc.dram_tensor("wk", [D, D], F32R, kind="ExternalInput")
    wv = nc.dram_tensor("wv", [D, D], F32R, kind="ExternalInput")
    w2 = nc.dram_tensor("w2", [D, D], F32R, kind="ExternalInput")
    b2 = nc.dram_tensor("b2", [1, D], F32R, kind="ExternalInput")
    out = nc.dram_tensor("out", [BPC, S, D], F32, kind="ExternalOutput")

    with tile.TileContext(nc) as tc:
        with ExitStack() as ctx:
            _emit(ctx, tc, nc, de, en, wq, wk, wv, w2, b2, out, repeat)
    nc.finalize()
    return nc


def _emit(ctx, tc, nc, de, en, wq, wk, wv, w2, b2, out, repeat=1):
    const = ctx.enter_context(tc.tile_pool(name="const", bufs=1))
    xtp = ctx.enter_context(tc.tile_pool(name="xtp", bufs=1))
    qkp = ctx.enter_context(tc.tile_pool(name="qkp", bufs=1))
    vsp = ctx.enter_context(tc.tile_pool(name="vsp", bufs=1))
    htp = ctx.enter_context(tc.tile_pool(name="htp", bufs=1))
    natp = ctx.enter_context(tc.tile_pool(name="natp", bufs=3))
    etp = ctx.enter_context(tc.tile_pool(name="etp", bufs=8))
    rowp = ctx.enter_context(tc.tile_pool(name="rowp", bufs=4))
    stgp = ctx.enter_context(tc.tile_pool(name="stgp", bufs=2))
    outp = ctx.enter_context(tc.tile_pool(name="outp", bufs=2))
    ps = ctx.enter_context(tc.tile_pool(name="ps", bufs=8, space="PSUM"))

    # --- one-time constants ---
    # Memset can't write fp32r directly (invalid ISA value type), so consts
    # are built in an fp32 scratch and rounded into fp32r via DVE copies.
    scr = const.tile([P, 3 * P], F32, tag="scr", name="scr")
    zeros_r = const.tile([P, 3 * P], F32R, tag="zeros", name="zeros_r")
    nc.gpsimd.memset(scr, 0.0)
    nc.vector.tensor_copy(zeros_r, scr)
    ident = const.tile([P, P], F32R, tag="ident", name="ident")
    make_identity(nc, scr[:, 0:P], nomemset=True)
    nc.vector.tensor_copy(ident, scr[:, 0:P])
    ones_t = const.tile([P, P], F32R, tag="ones", name="ones")
    nc.gpsimd.memset(scr[:, 0:P], 1.0)
    nc.vector.tensor_copy(ones_t, scr[:, 0:P])

    b2row = const.tile([1, D], F32R, tag="b2row", name="b2row")
    nc.sync.dma_start(b2row, b2[0:1, :])
    pb2 = ps.tile([P, D], F32, tag="ps", name="psmm")
    nc.tensor.matmul(pb2, (ones_t[0:1, 0:P]), (b2row), start=True, stop=True)
    b2t = const.tile([P, D], F32, tag="b2t", name="b2t")
    nc.scalar.copy(b2t, pb2)

    w_sb = {}
    for name, dram in (("wq", wq), ("wk", wk), ("wv", wv), ("w2", w2)):
        tiles = []
        for dt in range(NT):
            t = const.tile([P, D], F32R, tag=f"{name}{dt}", name=f"w_{name}{dt}")
            nc.sync.dma_start(t, dram[dt * P:(dt + 1) * P, :])
            tiles.append(t)
        w_sb[name] = tiles

    for b in [bb for _ in range(repeat) for bb in range(BPC)]:
        # --- phase A: load inputs and build x^T [d, s] ---
        xts = {}
        for name, dram in (("de", de), ("en", en)):
            xtbig = xtp.tile([P, NT * S], F32R, tag=f"{name}T", name=f"{name}T")
            xt = [xtbig[:, dt * S:(dt + 1) * S] for dt in range(NT)]
            for st in range(NT):
                natt = natp.tile([P, D], F32R, tag="nat", name="nat")
                nc.sync.dma_start(natt, dram[b, st * P:(st + 1) * P, :])
                pt = ps.tile([P, S], F32R, tag="ps", name="pst")
                for dt in range(NT):
                    nc.tensor.transpose(
                        pt[:, dt * P:(dt + 1) * P],
                        natt[:, dt * P:(dt + 1) * P],
                        ident,
                    )
                nc.vector.tensor_copy(
                    xtbig.rearrange("p (dt s) -> p dt s", s=S)[:, :, st * P:(st + 1) * P],
                    pt.rearrange("p (dt c) -> p dt c", c=P),
                )
            xts[name] = xt

        def qk_proj(xt, wname, tagpfx):
            dst = []
            for hp in range(4):
                pq = ps.tile([P, D], F32, tag="ps", name="psmm")
                for dt in range(NT):
                    nc.tensor.matmul(
                        pq,
                        (w_sb[wname][dt][:, hp * P:(hp + 1) * P]),
                        (xt[dt]),
                        start=dt == 0,
                        stop=dt == NT - 1,
                    )
                t = qkp.tile([P, D], F32R, tag=f"{tagpfx}{hp}", name=f"{tagpfx}{hp}")
                nc.vector.tensor_copy(t, pq)
                dst.append(t)
            return dst

        def v_proj(lhsT_tiles, tagpfx):
            # native [t, e] values for all heads; layout [128, 8*65] with a
            # ones column per head (for the softmax denominator)
            dst = []
            for tt in range(NT):
                pv = ps.tile([P, D], F32, tag="ps", name="psmm")
                for dt in range(NT):
                    nc.tensor.matmul(
                        pv,
                        (lhsT_tiles[dt][:, tt * P:(tt + 1) * P]),
                        (w_sb["wv"][dt]),
                        start=dt == 0,
                        stop=dt == NT - 1,
                    )
                t = vsp.tile([P, H * (DH + 1)], F32R, tag=f"{tagpfx}{tt}", name=f"{tagpfx}{tt}")
                dv = t.rearrange("p (h x) -> p h x", x=DH + 1)
                nc.scalar.copy(
                    dv[:, :, 0:DH], pv.rearrange("p (h e) -> p h e", e=DH)
                )
                nc.vector.tensor_copy(
                    dv[:, :, DH:DH + 1],
                    ones_t[:, 0:H].rearrange("p (h o) -> p h o", o=1),
                )
                dst.append(t)
            return dst

        def attn_head(h, qT, kT, v_s, hT, causal):
            hp, odd = divmod(h, 2)
            off = DH * odd
            e_tiles = []
            for ti in range(NT):
                s0 = ti * P if causal else 0
                pp = ps.tile([P, D], F32, tag="ps", name="psmm")
                nc.tensor.matmul(
                    pp[:, s0:D],
                    (kT[hp][off:off + DH, ti * P:(ti + 1) * P]),
                    (qT[hp][off:off + DH, s0:D]),
                    start=True,
                    stop=True,
                )
                et = etp.tile([P, D], F32R, tag="et", name="et")
                nc.scalar.activation(et[:, s0:D], pp[:, s0:D], EXP, scale=0.125)
                if causal:
                    nc.gpsimd.affine_select(
                        out=et[:, s0:s0 + P],
                        in_=et[:, s0:s0 + P],
                        compare_op=GE,
                        fill=0.0,
                        base=0,
                        pattern=[[1, P]],
                        channel_multiplier=-1,
                    )
                e_tiles.append(et)

            pa = ps.tile([DH + 1, D], F32, tag="ps", name="pa")
            for ti in range(NT):
                s0 = ti * P if causal else 0
                nc.tensor.matmul(
                    pa[:, s0:D],
                    (v_s[ti][:, h * (DH + 1):(h + 1) * (DH + 1)]),
                    (e_tiles[ti][:, s0:D]),
                    start=ti == 0,
                    stop=True,
                    skip_group_check=ti > 0,
                )
            # 1/Z broadcast across partitions via K=1 matmul
            rq = rowp.tile([DH + 1, D], F32R, tag="rq", name="rq")
            with nc.allow_low_precision("fp32r feeds the PE, full fp32 kept in PSUM"):
                nc.vector.reciprocal(rq[DH:DH + 1, :], pa[DH:DH + 1, :])
            prb = ps.tile([DH, D], F32, tag="ps", name="prb")
            nc.tensor.matmul(
                prb,
                (ones_t[DH:DH + 1, 0:DH]),
                (rq[DH:DH + 1, :]),
                start=True,
                stop=True,
            )
            # DVE can't read two PSUM operands (one PSUM read port) — evict
            # the broadcast to SBUF first
            prb_sb = stgp.tile([DH, D], F32, tag="prb", name="prb_sb")
            nc.scalar.copy(prb_sb, prb)
            if not odd:
                nc.vector.tensor_tensor(hT[hp][0:DH, :], pa[0:DH, :], prb_sb, MULT)
            else:
                stg = stgp.tile([DH, D], F32R, tag="stg", name="stg")
                nc.vector.tensor_tensor(stg, pa[0:DH, :], prb_sb, MULT)
                # partition shift (rows 0-63 -> 64-127) via SBUF->SBUF DMA
                nc.sync.dma_start(hT[hp][DH:P, :], stg)

        # --- block 1: masked self-attention on de_x ---
        q1T = qk_proj(xts["de"], "wq", "q1T")
        k1T = qk_proj(xts["de"], "wk", "k1T")
        v1s = v_proj(xts["de"], "v1s")
        h1T = [htp.tile([P, S], F32R, tag=f"h1T{dt}", name=f"h1T{dt}") for dt in range(NT)]
        for h in range(H):
            attn_head(h, q1T, k1T, v1s, h1T, causal=True)

        # --- block 2: same weights, q=k=en_x, v=h1 ---
        q2T = qk_proj(xts["en"], "wq", "q2T")
        k2T = qk_proj(xts["en"], "wk", "k2T")
        v2s = v_proj(h1T, "v2s")
        h2T = [htp.tile([P, S], F32R, tag=f"h2T{dt}", name=f"h2T{dt}") for dt in range(NT)]
        for h in range(H):
            attn_head(h, q2T, k2T, v2s, h2T, causal=False)

        # --- output projection + bias ---
        for st in range(NT):
            po = ps.tile([P, D], F32, tag="ps", name="psmm")
            for dt in range(NT):
                nc.tensor.matmul(
                    po,
                    (h2T[dt][:, st * P:(st + 1) * P]),
                    (w_sb["w2"][dt]),
                    start=dt == 0,
                    stop=dt == NT - 1,
                )
            ot = outp.tile([P, D], F32, tag="ot", name="ot")
            nc.vector.tensor_tensor(ot, po, b2t, ADD)
            nc.sync.dma_start(out[b, st * P:(st + 1) * P, :], ot)


def kernel(de_x, en_x, mask, Wq, Wk, Wv, W2, b2, _trace=False):
    de_x = np.asarray(de_x, dtype=np.float32)
    en_x = np.asarray(en_x, dtype=np.float32)
    # weights [H, D, DH] -> flat [D, H*DH]
    wqf = np.ascontiguousarray(np.transpose(np.asarray(Wq, np.float32), (1, 0, 2)).reshape(D, D))
    wkf = np.ascontiguousarray(np.transpose(np.asarray(Wk, np.float32), (1, 0, 2)).reshape(D, D))
    wvf = np.ascontiguousarray(np.transpose(np.asarray(Wv, np.float32), (1, 0, 2)).reshape(D, D))
    w2f = np.ascontiguousarray(np.asarray(W2, np.float32))
    b2f = np.ascontiguousarray(np.asarray(b2, np.float32).reshape(1, D))

    nc = _build()
    in_maps = []
    for c in range(NCORES):
        in_maps.append({
            "de_x": np.ascontiguousarray(de_x[c * BPC:(c + 1) * BPC]),
            "en_x": np.ascontiguousarray(en_x[c * BPC:(c + 1) * BPC]),
            "wq": wqf, "wk": wkf, "wv": wvf, "w2": w2f, "b2": b2f,
        })
    res = run_bass_kernel_spmd(nc, in_maps, list(range(NCORES)), trace=_trace)
    outs = np.concatenate([res.results[c]["out"] for c in range(NCORES)], axis=0)
    if _trace:
        return outs, res
    return outs


# revision 21
# speedup vs baseline: 10.6440x; 1.4700x over previous
"""Trainium2 Bass kernel for nn_DecoderLayer (B=16,S=512,D=512,H=8).

Sharding: pure data-parallel over batch. 16 batches / 8 cores = 2 per core.
Each core runs both attention blocks + output projection for its 2 batches.

Layout strategy (all matmuls keep the contraction dim on SBUF partitions):
  - inputs de_x/en_x are loaded natively then PE-transposed to x^T [d, s]
  - q^T/k^T computed per head-pair [128, 512] (two heads stacked on partitions)
  - v computed natively [t, e] for all heads at once (N=512 moving dim)
  - scores are computed transposed (p^T[t, s]) so softmax'd weights are
    directly usable as the moving operand of the PV matmul
  - softmax has no max-subtraction (scores ~ N(0,1), exp is safe in fp32);
    causal mask applied with affine_select (fill=0 after exp)
  - PV uses lhsT=[v | ones] so the softmax denominator Z lands in psum row 64
  - normalization (1/Z) is broadcast across partitions with a K=1 matmul
  - h1^T / h2^T are assembled per d-tile so the next projection can consume
    them directly as stationary operands (no extra transposes)
All matmul-feeding tensors are fp32r end-to-end (full PE rate at N>=256);
PSUM accumulation stays fp32.
"""

import numpy as np
from contextlib import ExitStack

import concourse.bacc as bacc
import concourse.bass as bass
import concourse.mybir as mybir
import concourse.tile as tile
from concourse.bass_utils import run_bass_kernel_spmd
from concourse.masks import make_identity

B, S, D, H = 16, 512, 512, 8
DH = D // H              # 64
NCORES = 8
BPC = B // NCORES        # 2 batches per core
P = 128
NT = S // P              # 4 tiles along s/t/d
F32 = mybir.dt.float32
F32R = mybir.dt.float32r
EXP = mybir.ActivationFunctionType.Exp
MULT = mybir.AluOpType.mult
ADD = mybir.AluOpType.add
GE = mybir.AluOpType.is_ge


def _build(repeat=1):
    nc = bacc.Bacc("TRN2", target_bir_lowering=False)
    de = nc.dram_tensor("de_x", [BPC, S, D], F32R, kind="ExternalInput")
    en = nc.dram_tensor("en_x", [BPC, S, D], F32R, kind="ExternalInput")
    wq = nc.dram_tensor("wq", [D, D], F32R, kind="ExternalInput")
    wk = nc.dram_tensor("wk", [D, D], F32R, kind="ExternalInput")
    wv = nc.dram_tensor("wv", [D, D], F32R, kind="ExternalInput")
    w2 = nc.dram_tensor("w2", [D, D], F32R, kind="ExternalInput")
    b2 = nc.dram_tensor("b2", [1, D], F32R, kind="ExternalInput")
    out = nc.dram_tensor("out", [BPC, S, D], F32, kind="ExternalOutput")

    with tile.TileContext(nc) as tc:
        with ExitStack() as ctx:
            _emit(ctx, tc, nc, de, en, wq, wk, wv, w2, b2, out, repeat)
    nc.finalize()
    return nc


def _emit(ctx, tc, nc, de, en, wq, wk, wv, w2, b2, out, repeat=1):
    const = ctx.enter_context(tc.tile_pool(name="const", bufs=1))
    xtp = ctx.enter_context(tc.tile_pool(name="xtp", bufs=2))
    qkp = ctx.enter_context(tc.tile_pool(name="qkp", bufs=1))
    vsp = ctx.enter_context(tc.tile_pool(name="vsp", bufs=1))
    htp = ctx.enter_context(tc.tile_pool(name="htp", bufs=1))
    natp = ctx.enter_context(tc.tile_pool(name="natp", bufs=3))
    etp = ctx.enter_context(tc.tile_pool(name="etp", bufs=8))
    rowp = ctx.enter_context(tc.tile_pool(name="rowp", bufs=4))
    stgp = ctx.enter_context(tc.tile_pool(name="stgp", bufs=2))
    outp = ctx.enter_context(tc.tile_pool(name="outp", bufs=2))
    ps = ctx.enter_context(tc.tile_pool(name="ps", bufs=2, space="PSUM"))
    ppp = ctx.enter_context(tc.tile_pool(name="ppp", bufs=2, space="PSUM"))
    ptp = ctx.enter_context(tc.tile_pool(name="ptp", bufs=1, space="PSUM"))
    pap = ctx.enter_context(tc.tile_pool(name="pap", bufs=2, space="PSUM"))
    prp = ctx.enter_context(tc.tile_pool(name="prp", bufs=1, space="PSUM"))

    # --- one-time constants ---
    # Memset can't write fp32r directly (invalid ISA value type), so consts
    # are built in an fp32 scratch and rounded into fp32r via DVE copies.
    scr = const.tile([P, 3 * P], F32, tag="scr", name="scr")
    zeros_r = const.tile([P, 3 * P], F32R, tag="zeros", name="zeros_r")
    nc.gpsimd.memset(scr, 0.0)
    nc.vector.tensor_copy(zeros_r, scr)
    ident = const.tile([P, P], F32R, tag="ident", name="ident")
    make_identity(nc, scr[:, 0:P], nomemset=True)
    nc.vector.tensor_copy(ident, scr[:, 0:P])
    ones_t = const.tile([P, P], F32R, tag="ones", name="ones")
    nc.gpsimd.memset(scr[:, 0:P], 1.0)
    nc.vector.tensor_copy(ones_t, scr[:, 0:P])

    b2row = const.tile([1, D], F32R, tag="b2row", name="b2row")
    nc.sync.dma_start(b2row, b2[0:1, :])
    pb2 = ps.tile([P, D], F32, tag="ps", name="psmm")
    nc.tensor.matmul(pb2, (ones_t[0:1, 0:P]), (b2row), start=True, stop=True)
    b2t = const.tile([P, D], F32, tag="b2t", name="b2t")
    nc.scalar.copy(b2t, pb2)

    w_sb = {}
    for name, dram in (("wq", wq), ("wk", wk), ("wv", wv), ("w2", w2)):
        tiles = []
        for dt in range(NT):
            t = const.tile([P, D], F32R, tag=f"{name}{dt}", name=f"w_{name}{dt}")
            nc.gpsimd.dma_start(t, dram[dt * P:(dt + 1) * P, :])
            tiles.append(t)
        w_sb[name] = tiles

    for b in [bb for _ in range(repeat) for bb in range(BPC)]:
        # --- phase A: load inputs and build x^T [d, s] ---
        xts = {}
        for name, dram in (("de", de), ("en", en)):
            xtbig = xtp.tile([P, NT * S], F32R, tag=f"{name}T", name=f"{name}T")
            xt = [xtbig[:, dt * S:(dt + 1) * S] for dt in range(NT)]
            for st in range(NT):
                natt = natp.tile([P, D], F32R, tag="nat", name="nat")
                nc.sync.dma_start(natt, dram[b, st * P:(st + 1) * P, :])
                pt = ptp.tile([P, S], F32R, tag="pt", name="pst")
                for dt in range(NT):
                    nc.tensor.transpose(
                        pt[:, dt * P:(dt + 1) * P],
                        natt[:, dt * P:(dt + 1) * P],
                        ident,
                    )
                nc.vector.tensor_copy(
                    xtbig.rearrange("p (dt s) -> p dt s", s=S)[:, :, st * P:(st + 1) * P],
                    pt.rearrange("p (dt c) -> p dt c", c=P),
                )
            xts[name] = xt

        def qk_proj(xt, wname, tagpfx):
            dst = []
            for hp in range(4):
                pq = ps.tile([P, D], F32, tag="ps", name="psmm")
                for dt in range(NT):
                    nc.tensor.matmul(
                        pq,
                        (w_sb[wname][dt][:, hp * P:(hp + 1) * P]),
                        (xt[dt]),
                        start=dt == 0,
                        stop=dt == NT - 1,
                    )
                t = qkp.tile([P, D], F32R, tag=f"{tagpfx}{hp}", name=f"{tagpfx}{hp}")
                nc.vector.tensor_copy(t, pq)
                dst.append(t)
            return dst

        def v_proj(lhsT_tiles, tagpfx):
            # native [t, e] values for all heads; layout [128, 8*65] with a
            # ones column per head (for the softmax denominator)
            dst = []
            for tt in range(NT):
                pv = ps.tile([P, D], F32, tag="ps", name="psmm")
                for dt in range(NT):
                    nc.tensor.matmul(
                        pv,
                        (lhsT_tiles[dt][:, tt * P:(tt + 1) * P]),
                        (w_sb["wv"][dt]),
                        start=dt == 0,
                        stop=dt == NT - 1,
                    )
                t = vsp.tile([P, H * (DH + 1)], F32R, tag=f"{tagpfx}{tt}", name=f"{tagpfx}{tt}")
                dv = t.rearrange("p (h x) -> p h x", x=DH + 1)
                nc.vector.tensor_copy(
                    dv[:, :, 0:DH], pv.rearrange("p (h e) -> p h e", e=DH)
                )
                nc.vector.tensor_copy(
                    dv[:, :, DH:DH + 1],
                    ones_t[:, 0:H].rearrange("p (h o) -> p h o", o=1),
                )
                dst.append(t)
            return dst

        def attn_head(h, qT, kT, v_s, hT, causal):
            hp, odd = divmod(h, 2)
            off = DH * odd
            e_tiles = []
            for ti in range(NT):
                s0 = ti * P if causal else 0
                m0 = min(s0, D - 2 * P)  # keep moving dim >=256 (fp32r full rate)
                pp = ppp.tile([P, D], F32, tag="pp", name="pp")
                nc.tensor.matmul(
                    pp[:, m0:D],
                    (kT[hp][off:off + DH, ti * P:(ti + 1) * P]),
                    (qT[hp][off:off + DH, m0:D]),
                    start=True,
                    stop=True,
                )
                et = etp.tile([P, D], F32R, tag="et", name="et")
                nc.scalar.activation(et[:, s0:D], pp[:, s0:D], EXP, scale=0.125)
                if causal:
                    nc.gpsimd.affine_select(
                        out=et[:, s0:s0 + P],
                        in_=et[:, s0:s0 + P],
                        compare_op=GE,
                        fill=0.0,
                        base=0,
                        pattern=[[1, P]],
                        channel_multiplier=-1,
                    )
                e_tiles.append(et)

            pa = pap.tile([DH + 1, D], F32, tag="pa", name="pa")
            for ti in range(NT):
                s0 = ti * P if causal else 0
                nc.tensor.matmul(
                    pa[:, s0:D],
                    (v_s[ti][:, h * (DH + 1):(h + 1) * (DH + 1)]),
                    (e_tiles[ti][:, s0:D]),
                    start=ti == 0,
                    stop=True,
                    skip_group_check=ti > 0,
                )
            # 1/Z broadcast across partitions via K=1 matmul
            rq = rowp.tile([DH + 1, D], F32R, tag="rq", name="rq")
            with nc.allow_low_precision("fp32r feeds the PE, full fp32 kept in PSUM"):
                nc.vector.reciprocal(rq[DH:DH + 1, :], pa[DH:DH + 1, :])
            prb = prp.tile([DH, D], F32, tag="prb", name="prb")
            nc.tensor.matmul(
                prb,
                (ones_t[DH:DH + 1, 0:DH]),
                (rq[DH:DH + 1, :]),
                start=True,
                stop=True,
            )
            # DVE can't read two PSUM operands (one PSUM read port) — evict
            # the broadcast to SBUF first
            prb_sb = stgp.tile([DH, D], F32, tag="prb", name="prb_sb")
            nc.scalar.copy(prb_sb, prb)
            if not odd:
                nc.vector.tensor_tensor(hT[hp][0:DH, :], pa[0:DH, :], prb_sb, MULT)
            else:
                stg = stgp.tile([DH, D], F32R, tag="stg", name="stg")
                nc.vector.tensor_tensor(stg, pa[0:DH, :], prb_sb, MULT)
                # partition shift (rows 0-63 -> 64-127) via SBUF->SBUF DMA
                nc.sync.dma_start(hT[hp][DH:P, :], stg)

        # --- block 1: masked self-attention on de_x ---
        q1T = qk_proj(xts["de"], "wq", "q1T")
        k1T = qk_proj(xts["de"], "wk", "k1T")
        v1s = v_proj(xts["de"], "v1s")
        h1T = [htp.tile([P, S], F32R, tag=f"h1T{dt}", name=f"h1T{dt}") for dt in range(NT)]
        for h in (1, 3, 5, 7, 0, 2, 4, 6):
            attn_head(h, q1T, k1T, v1s, h1T, causal=True)

        # --- block 2: same weights, q=k=en_x, v=h1 ---
        q2T = qk_proj(xts["en"], "wq", "q2T")
        k2T = qk_proj(xts["en"], "wk", "k2T")
        v2s = v_proj(h1T, "v2s")
        h2T = [htp.tile([P, S], F32R, tag=f"h2T{dt}", name=f"h2T{dt}") for dt in range(NT)]
        for h in (1, 3, 5, 7, 0, 2, 4, 6):
            attn_head(h, q2T, k2T, v2s, h2T, causal=False)

        # --- output projection + bias ---
        for st in range(NT):
            po = ps.tile([P, D], F32, tag="ps", name="psmm")
            for dt in range(NT):
                nc.tensor.matmul(
                    po,
                    (h2T[dt][:, st * P:(st + 1) * P]),
                    (w_sb["w2"][dt]),
                    start=dt == 0,
                    stop=dt == NT - 1,
                )
            ot = outp.tile([P, D], F32, tag="ot", name="ot")
            nc.vector.tensor_tensor(ot, po, b2t, ADD)
            nc.gpsimd.dma_start(out[b, st * P:(st + 1) * P, :], ot)


def kernel(de_x, en_x, mask, Wq, Wk, Wv, W2, b2, _trace=False):
    de_x = np.asarray(de_x, dtype=np.float32)
    en_x = np.asarray(en_x, dtype=np.float32)
    # weights [H, D, DH] -> flat [D, H*DH]
    wqf = np.ascontiguousarray(np.transpose(np.asarray(Wq, np.float32), (1, 0, 2)).reshape(D, D))
    wkf = np.ascontiguousarray(np.transpose(np.asarray(Wk, np.float32), (1, 0, 2)).reshape(D, D))
    wvf = np.ascontiguousarray(np.transpose(np.asarray(Wv, np.float32), (1, 0, 2)).reshape(D, D))
    w2f = np.ascontiguousarray(np.asarray(W2, np.float32))
    b2f = np.ascontiguousarray(np.asarray(b2, np.float32).reshape(1, D))

    nc = _build()
    in_maps = []
    for c in range(NCORES):
        in_maps.append({
            "de_x": np.ascontiguousarray(de_x[c * BPC:(c + 1) * BPC]),
            "en_x": np.ascontiguousarray(en_x[c * BPC:(c + 1) * BPC]),
            "wq": wqf, "wk": wkf, "wv": wvf, "w2": w2f, "b2": b2f,
        })
    res = run_bass_kernel_spmd(nc, in_maps, list(range(NCORES)), trace=_trace)
    outs = np.concatenate([res.results[c]["out"] for c in range(NCORES)], axis=0)
    if _trace:
        return outs, res
    return outs
